# revision 15
# baseline (speedup 1.0000x reference)
"""Trainium2 Bass kernel for GCNCriticNet (gnn_message_passing).

Graphs are 8192 independent complete graphs of 16 nodes (+ self loops): every
node has degree 16, the symmetric norm is 1/16, and GCN aggregation collapses
to a per-graph mean. Edge lists never reach the device.

Per core (16384 nodes = 1024 graphs), feature-major [128, node-cols], node
columns ordered (s, g) — node-within-graph major — within each layout unit.
Macros 0-6 are one 2048-col unit (gpm=128); macro 7 is two 1024-col units
(gpm=64) so the tail dependency chain is halved. Per unit:
  u1 = Wcomb^T Z       Z = [obs ; bcast(graph-sum obs)] (K=128 stacked), PSUM
  x1 = tanh(u1 + b1f)  ACT -> SBUF bf16
  sx1 = group-sum(x1)  DVE pairwise tree, bf16 2x mode
  u2  = x1 + W2s^T sx1 rebuilt in PSUM by PE: identity-matmul of x1
                       (start=True) + accumulate matmuls with a stride-0
                       broadcast rhs (start=False) -> no broadcast DMA, no
                       DVE adds, f32 accumulation
  x2  = tanh(u2 + b2)  ACT (b2 via ACT bias) -> SBUF bf16
  sx2 = group-sum(x2)  DVE tree -> slice of sx2all
Output: sx2all [128, 1024] bf16 DMA'd out in 3 pieces; host applies the tiny
wfc^T matvec + b_fc1 (mean's /16 folded into the weights).

Startup: the critical first DMAs are spread across the sync/scalar/gpsimd
queues (each dma_start costs ~600ns serialized on its issuing queue), macro
0's u1/tanh1 run in halves so ACT starts early, and a 1-col dummy tanh
preloads the ACT table set while the first obs chunk is in flight.
"""

import sys
import numpy as np

try:
    import concourse.bass as bass  # noqa: F401
except ImportError:  # harness runs in a bare dir; repo is on the box
    for p in ("/opt/trn_rl_repo", "/root/.axon_site/_ro/trn_rl_repo"):
        if p not in sys.path:
            sys.path.insert(0, p)
    import concourse.bass as bass  # noqa: F401

import ml_dtypes
import concourse.bacc as bacc
import concourse.mybir as mybir
import concourse.tile as tile
from concourse.bass import MemorySpace
from concourse.bass_utils import run_bass_kernel_spmd

F32 = mybir.dt.float32
BF16 = mybir.dt.bfloat16
AF = mybir.ActivationFunctionType

N_CORES = 8
N_AGENTS = 16
BATCH = 8192
OBS = 64
HID = 128
N = BATCH * N_AGENTS            # 131072 nodes
NPC = N // N_CORES              # 16384 nodes / core
MC = 2048                       # nodes per macro-chunk
NMC = NPC // MC                 # 8
OUTPC = NPC // N_AGENTS         # 1024 graphs per core
S = N_AGENTS

# layout units per macro: (col offset within macro, width, graphs per unit)
def _units(m):
    if m == NMC - 1:
        return [(0, 1024, 64), (1024, 1024, 64)]
    return [(0, MC, 128)]

# sx2 column offset of each macro
_SXOFF = [0]
for _m in range(1, NMC + 1):
    _SXOFF.append(_SXOFF[-1] + MC // S)

_CACHE = {}


def _build_nc():
    nc = bacc.Bacc("TRN2", target_bir_lowering=False, debug=False)

    obs_d = nc.dram_tensor("obs", [NMC, 128, MC], BF16, kind="ExternalInput")
    wca_d = nc.dram_tensor("wca", [128, HID], BF16, kind="ExternalInput")
    wpk_d = nc.dram_tensor("wpk", [128, 2 * HID], BF16, kind="ExternalInput")
    bpk_d = nc.dram_tensor("bpk", [128, 2], F32, kind="ExternalInput")
    # per-graph sums of x2; host applies wfc^T (tiny matvec) + b_fc1
    out_d = nc.dram_tensor("out", [128, OUTPC], BF16, kind="ExternalOutput")

    with tile.TileContext(nc) as tc:
        with (
            tc.tile_pool(name="const", bufs=1) as cp,
            tc.tile_pool(name="zt", bufs=3) as ztp,
            tc.tile_pool(name="sc", bufs=2) as scp,
            tc.tile_pool(name="x1p", bufs=2) as x1p,
            tc.tile_pool(name="x2p", bufs=2) as x2p,
            tc.tile_pool(name="pup", bufs=1, space=MemorySpace.PSUM) as pup,
        ):
            wsb = cp.tile([128, 3 * HID], BF16)
            bsb = cp.tile([128, 2], F32)
            sx2all = cp.tile([128, OUTPC], BF16)
            dumt = cp.tile([128, 1], BF16)

            wcomb = wsb[:, 0:HID]
            w2s = wsb[:, HID:2 * HID]
            ident = wsb[:, 2 * HID:3 * HID]
            b1f = bsb[:, 0:1]
            b2 = bsb[:, 1:2]

            zt_of, u1_of, x1_of, u2_of, x2_of = {}, {}, {}, {}, {}

            # startup: spread the critical first DMAs across three queues so
            # their ~600ns issue costs overlap, and preload the tanh table.
            zt0 = ztp.tile([128, MC], BF16, tag="zt")
            zt_of[0] = zt0
            nc.sync.dma_start(wsb[:, 0:HID], wca_d[:])
            nc.sync.dma_start(zt0[:, 0:1024], obs_d[0, :, 0:1024])
            nc.sync.dma_start(zt0[:, 1024:MC], obs_d[0, :, 1024:MC])
            nc.scalar.activation(dumt[:], dumt[:], AF.Tanh)
            nc.gpsimd.dma_start(bsb[:], bpk_d[:])

            def stage_a(m):
                zt = ztp.tile([128, MC], BF16, tag="zt")
                zt_of[m] = zt
                nc.sync.dma_start(zt[:], obs_d[m])
                if m == 1:  # rest of the weights, needed first at e(0)
                    nc.sync.dma_start(wsb[:, HID:3 * HID], wpk_d[:])

            def tree16(src_ap, dst_ap, width, tag):
                """Contiguous pairwise s-tree: src [128, 16*w] -> dst [128, w]."""
                t = f"{tag}{width}"
                a = scp.tile([128, 8 * width], BF16, tag=t + "a")
                nc.vector.tensor_add(a[:], src_ap[:, 0:8 * width],
                                     src_ap[:, 8 * width:16 * width])
                b = scp.tile([128, 4 * width], BF16, tag=t + "b")
                nc.vector.tensor_add(b[:], a[:, 0:4 * width], a[:, 4 * width:8 * width])
                c = scp.tile([128, 2 * width], BF16, tag=t + "c")
                nc.vector.tensor_add(c[:], b[:, 0:2 * width], b[:, 2 * width:4 * width])
                nc.vector.tensor_add(dst_ap, c[:, 0:width], c[:, width:2 * width])

            def stage_bc(m):
                zt = zt_of.pop(m)
                u1 = pup.tile([128, MC], F32, tag="u1")
                x1 = x1p.tile([128, MC], BF16, tag="x1")
                x1_of[m] = x1
                split = 2 if m == 0 else 1
                w = MC // split
                for h in range(split):
                    o0 = h * w
                    for o in range(o0, o0 + w, 512):
                        nc.tensor.matmul(u1[:, o:o + 512], wcomb,
                                         zt[:, o:o + 512], start=True, stop=True)
                    nc.scalar.activation(x1[:, o0:o0 + w], u1[:, o0:o0 + w],
                                         AF.Tanh, bias=b1f)

            def stage_de(m, uo, uw, gpm):
                """tree1 + u2 rebuild in PSUM for one layout unit."""
                x1 = x1_of[m]
                if uo == 0:
                    # the last macro reuses the u1 slot (free after its tanh1):
                    # avoids the WAR wait on tanh2(m-1) draining the u2 slot
                    utag = "u1" if m == NMC - 1 else "u2"
                    u2_of[m] = pup.tile([128, MC], F32, tag=utag, name="u2")
                u2 = u2_of[m]
                sx1 = scp.tile([128, gpm], BF16, tag=f"sx1{gpm}")
                tree16(x1[:, uo:uo + uw], sx1[:], gpm, "s1")
                spb = 512 // gpm  # s-slots per PSUM bank
                sx1b = sx1[:].rearrange("p (o g) -> p o g", o=1).broadcast_to(
                    [128, spb, gpm])
                for o in range(uo, uo + uw, 512):
                    nc.tensor.matmul(u2[:, o:o + 512], ident, x1[:, o:o + 512],
                                     start=True, stop=False)
                    nc.tensor.matmul(
                        u2[:, o:o + 512].rearrange("p (s g) -> p s g", s=spb),
                        w2s, sx1b, start=False, stop=True)
                if uo + uw == MC:
                    x1_of.pop(m)

            def stage_f(m, uo, uw):
                u2 = u2_of[m]
                if uo == 0:
                    x2_of[m] = x2p.tile([128, MC], BF16, tag="x2", name="x2")
                x2 = x2_of[m]
                nc.scalar.activation(x2[:, uo:uo + uw], u2[:, uo:uo + uw],
                                     AF.Tanh, bias=b2)
                if uo + uw == MC:
                    u2_of.pop(m)

            def stage_g(m, uo, uw, gpm):
                x2 = x2_of[m]
                so = _SXOFF[m] + uo // S
                tree16(x2[:, uo:uo + uw], sx2all[:, so:so + gpm], gpm, "s2")
                if uo + uw == MC:
                    x2_of.pop(m)

            # software pipeline at layout-unit granularity; tree2 of the
            # previous unit is issued AFTER tree1/u2 of the current one so the
            # in-order DVE queue never blocks the forward chain.
            UNITS = [(m, uo, uw, gpm) for m in range(NMC)
                     for uo, uw, gpm in _units(m)]
            stage_a(1)
            stage_a(2)
            stage_bc(0)
            prev = None
            for k, (m, uo, uw, gpm) in enumerate(UNITS):
                if uo == 0:
                    if m + 3 < NMC:
                        stage_a(m + 3)
                    if m + 1 < NMC:
                        stage_bc(m + 1)
                stage_de(m, uo, uw, gpm)
                if prev is not None:
                    stage_g(*prev)
                    pm = prev[0]
                    if prev[1] + prev[2] == MC:
                        if pm == 3:
                            nc.sync.dma_start(out_d[:, 0:512], sx2all[:, 0:512])
                        elif pm == 6:
                            nc.sync.dma_start(out_d[:, 512:896],
                                              sx2all[:, 512:896])
                    elif pm == NMC - 1:  # first unit of the last macro
                        nc.sync.dma_start(out_d[:, 896:960], sx2all[:, 896:960])
                stage_f(m, uo, uw)
                prev = (m, uo, uw, gpm)
            stage_g(*prev)
            nc.sync.dma_start(out_d[:, 960:OUTPC], sx2all[:, 960:OUTPC])

    nc.compile()
    return nc


def _get_nc():
    if "nc" not in _CACHE:
        _CACHE["nc"] = _build_nc()
    return _CACHE["nc"]


def _pack_block(o4):
    """[gpm, 16, 64] float32 node block -> [128, 16*gpm] bf16 device block."""
    gpm = o4.shape[0]
    top = o4.transpose(2, 1, 0)                        # [OBS, S, gpm]
    sob = o4.sum(axis=1).transpose(1, 0)               # [OBS, gpm]
    bot = np.broadcast_to(sob[:, None, :], top.shape)
    blk = np.concatenate([top, bot], axis=0)           # [128, S, gpm]
    return blk.reshape(128, S * gpm)


def _make_in_maps(cent_obs, w_emb, b_emb, w_gcn, b_gcn):
    w_emb = np.ascontiguousarray(w_emb, np.float32)
    wcomb = np.concatenate(
        [w_emb, (w_emb @ w_gcn[0]) / np.float32(16.0)], axis=0
    )                                                      # [128, HID]
    w2s = w_gcn[1] / np.float32(16.0)                      # [HID, HID]
    ident = np.eye(HID, dtype=np.float32)
    wca = np.ascontiguousarray(wcomb).astype(ml_dtypes.bfloat16)
    wpk = np.concatenate([w2s, ident],
                         axis=1).astype(ml_dtypes.bfloat16)  # [128, 256]
    b1f = (b_gcn[0] + b_emb + b_emb @ w_gcn[0]).astype(np.float32).reshape(HID, 1)
    b2 = b_gcn[1].astype(np.float32).reshape(HID, 1)
    bpk = np.concatenate([b1f, b2], axis=1).astype(np.float32)  # [128, 2]
    shared = {"wca": wca, "wpk": wpk, "bpk": bpk}
    o5 = np.ascontiguousarray(cent_obs, np.float32).reshape(
        N_CORES, NMC, MC // S, S, OBS
    )
    obs_all = np.zeros((N_CORES, NMC, 128, MC), np.float32)
    for m in range(NMC):
        for uo, uw, gpm in _units(m):
            g0 = uo // S
            obs_all[:, m, :, uo:uo + uw] = np.stack(
                [_pack_block(o5[ci, m, g0:g0 + gpm]) for ci in range(N_CORES)]
            )
    obs_all = obs_all.astype(ml_dtypes.bfloat16)
    in_maps = []
    for ci in range(N_CORES):
        m = dict(shared)
        m["obs"] = np.ascontiguousarray(obs_all[ci])
        in_maps.append(m)
    return in_maps


def kernel(cent_obs, w_emb, b_emb, w_gcn, b_gcn, w_fc1, b_fc1,
           edge_src, edge_dst, _trace=False):
    cent_obs = np.asarray(cent_obs, np.float32)
    nc = _get_nc()
    in_maps = _make_in_maps(
        cent_obs, np.asarray(w_emb, np.float32), np.asarray(b_emb, np.float32),
        np.asarray(w_gcn, np.float32), np.asarray(b_gcn, np.float32),
    )
    kw = dict(trace=True) if _trace else {}
    res = run_bass_kernel_spmd(nc, in_maps, list(range(N_CORES)), **kw)
    wfc = (np.asarray(w_fc1, np.float32).reshape(HID) / np.float32(16.0))
    y = np.concatenate(
        [wfc @ np.asarray(res.results[i]["out"]).astype(np.float32)
         for i in range(N_CORES)]
    )
    out = (y + np.float32(np.asarray(b_fc1).reshape(()))).astype(np.float32)
    if _trace:
        _CACHE["last_result"] = res
    return out.reshape(BATCH, 1)


# revision 17
# speedup vs baseline: 1.2017x; 1.2017x over previous
"""Trainium2 Bass kernel for GCNCriticNet (gnn_message_passing).

Graphs are 8192 independent complete graphs of 16 nodes (+ self loops): every
node has degree 16, the symmetric norm is 1/16, and GCN aggregation collapses
to a per-graph mean. Edge lists never reach the device.

Per core (16384 nodes = 1024 graphs), feature-major [128, node-cols], node
columns ordered (s, g) — node-within-graph major — within each layout unit.
Macros 0-6 are one 2048-col unit (gpm=128); macro 7 is two 1024-col units
(gpm=64) so the tail dependency chain is halved. Per unit:
  u1 = Wcomb^T Z       Z = [obs ; bcast(graph-sum obs)] (K=128 stacked), PSUM
  x1 = tanh(u1 + b1f)  ACT -> SBUF bf16
  sx1 = group-sum(x1)  DVE pairwise tree, bf16 2x mode
  u2  = x1 + W2s^T sx1 rebuilt in PSUM by PE: identity-matmul of x1
                       (start=True) + accumulate matmuls with a stride-0
                       broadcast rhs (start=False) -> no broadcast DMA, no
                       DVE adds, f32 accumulation
  x2  = tanh(u2 + b2)  ACT (b2 via ACT bias) -> SBUF bf16
  sx2 = group-sum(x2)  DVE tree -> slice of sx2all
Output: sx2all [128, 1024] bf16 DMA'd out in 3 pieces; host applies the tiny
wfc^T matvec + b_fc1 (mean's /16 folded into the weights).

Startup: the critical first DMAs are spread across the sync/scalar/gpsimd
queues (each dma_start costs ~600ns serialized on its issuing queue), macro
0's u1/tanh1 run in halves so ACT starts early, and a 1-col dummy tanh
preloads the ACT table set while the first obs chunk is in flight.
"""

import sys
import numpy as np

try:
    import concourse.bass as bass  # noqa: F401
except ImportError:  # harness runs in a bare dir; repo is on the box
    for p in ("/opt/trn_rl_repo", "/root/.axon_site/_ro/trn_rl_repo"):
        if p not in sys.path:
            sys.path.insert(0, p)
    import concourse.bass as bass  # noqa: F401

import ml_dtypes
import concourse.bacc as bacc
import concourse.mybir as mybir
import concourse.tile as tile
from concourse.bass import MemorySpace
from concourse.bass_utils import run_bass_kernel_spmd

F32 = mybir.dt.float32
BF16 = mybir.dt.bfloat16
AF = mybir.ActivationFunctionType

N_CORES = 8
N_AGENTS = 16
BATCH = 8192
OBS = 64
HID = 128
N = BATCH * N_AGENTS            # 131072 nodes
NPC = N // N_CORES              # 16384 nodes / core
MC = 2048                       # nodes per macro-chunk
NMC = NPC // MC                 # 8
OUTPC = NPC // N_AGENTS         # 1024 graphs per core
S = N_AGENTS

# layout units per macro: (col offset within macro, width, graphs per unit)
def _units(m):
    if m == NMC - 1:
        return [(0, 1024, 64), (1024, 1024, 64)]
    return [(0, MC, 128)]

# sx2 column offset of each macro
_SXOFF = [0]
for _m in range(1, NMC + 1):
    _SXOFF.append(_SXOFF[-1] + MC // S)

_CACHE = {}


def _build_nc():
    nc = bacc.Bacc("TRN2", target_bir_lowering=False, debug=False)

    obs_d = nc.dram_tensor("obs", [NMC, 128, MC], BF16, kind="ExternalInput")
    wca_d = nc.dram_tensor("wca", [128, HID], BF16, kind="ExternalInput")
    wpk_d = nc.dram_tensor("wpk", [128, 2 * HID], BF16, kind="ExternalInput")
    bpk_d = nc.dram_tensor("bpk", [128, 2], F32, kind="ExternalInput")
    # per-graph sums of x2; host applies wfc^T (tiny matvec) + b_fc1
    out_d = nc.dram_tensor("out", [128, OUTPC], BF16, kind="ExternalOutput")

    with tile.TileContext(nc) as tc:
        with (
            tc.tile_pool(name="const", bufs=1) as cp,
            tc.tile_pool(name="zt", bufs=3) as ztp,
            tc.tile_pool(name="sc", bufs=2) as scp,
            tc.tile_pool(name="x1p", bufs=2) as x1p,
            tc.tile_pool(name="x2p", bufs=2) as x2p,
            tc.tile_pool(name="pup", bufs=1, space=MemorySpace.PSUM) as pup,
        ):
            wsb = cp.tile([128, 3 * HID], BF16)
            bsb = cp.tile([128, 2], F32)
            sx2all = cp.tile([128, OUTPC], BF16)
            dumt = cp.tile([128, 1], BF16)

            wcomb = wsb[:, 0:HID]
            w2s = wsb[:, HID:2 * HID]
            ident = wsb[:, 2 * HID:3 * HID]
            b1f = bsb[:, 0:1]
            b2 = bsb[:, 1:2]

            zt_of, u1_of, x1_of, u2_of, x2_of = {}, {}, {}, {}, {}

            # startup: spread the critical first DMAs across three queues so
            # their ~600ns issue costs overlap, and preload the tanh table.
            zt0 = ztp.tile([128, MC], BF16, tag="zt")
            zt_of[0] = zt0
            nc.sync.dma_start(wsb[:, 0:HID], wca_d[:])
            nc.sync.dma_start(zt0[:, 0:1024], obs_d[0, :, 0:1024])
            nc.sync.dma_start(zt0[:, 1024:MC], obs_d[0, :, 1024:MC])
            nc.scalar.activation(dumt[:], dumt[:], AF.Tanh)
            nc.gpsimd.dma_start(bsb[:], bpk_d[:])

            def stage_a(m):
                zt = ztp.tile([128, MC], BF16, tag="zt")
                zt_of[m] = zt
                nc.sync.dma_start(zt[:], obs_d[m])
                if m == 1:  # rest of the weights, needed first at e(0)
                    nc.sync.dma_start(wsb[:, HID:3 * HID], wpk_d[:])

            def tree16(src_ap, dst_ap, width, tag):
                """Contiguous pairwise s-tree: src [128, 16*w] -> dst [128, w]."""
                t = f"{tag}{width}"
                a = scp.tile([128, 8 * width], BF16, tag=t + "a")
                nc.vector.tensor_add(a[:], src_ap[:, 0:8 * width],
                                     src_ap[:, 8 * width:16 * width])
                b = scp.tile([128, 4 * width], BF16, tag=t + "b")
                nc.vector.tensor_add(b[:], a[:, 0:4 * width], a[:, 4 * width:8 * width])
                c = scp.tile([128, 2 * width], BF16, tag=t + "c")
                nc.vector.tensor_add(c[:], b[:, 0:2 * width], b[:, 2 * width:4 * width])
                nc.vector.tensor_add(dst_ap, c[:, 0:width], c[:, width:2 * width])

            def stage_bc(m):
                zt = zt_of.pop(m)
                u1 = pup.tile([128, MC], F32, tag="uA" if m == 0 else "uB")
                x1 = x1p.tile([128, MC], BF16, tag="x1")
                x1_of[m] = x1
                split = 2 if m == 0 else 1
                w = MC // split
                for h in range(split):
                    o0 = h * w
                    for o in range(o0, o0 + w, 512):
                        nc.tensor.matmul(u1[:, o:o + 512], wcomb,
                                         zt[:, o:o + 512], start=True, stop=True)
                    nc.scalar.activation(x1[:, o0:o0 + w], u1[:, o0:o0 + w],
                                         AF.Tanh, bias=b1f)

            def stage_de(m, uo, uw, gpm):
                """tree1 + u2 rebuild in PSUM for one layout unit."""
                x1 = x1_of[m]
                if uo == 0:
                    # u2(0..6) share slot A with u1(0); u2(7) follows u1(7)
                    # in slot B -- each u2 only waits a drained read, never a
                    # tanh2 of another macro, at both ends of the pipeline
                    utag = "uB" if m == NMC - 1 else "uA"
                    u2_of[m] = pup.tile([128, MC], F32, tag=utag, name="u2")
                u2 = u2_of[m]
                sx1 = scp.tile([128, gpm], BF16, tag=f"sx1{gpm}")
                tree16(x1[:, uo:uo + uw], sx1[:], gpm, "s1")
                spb = 512 // gpm  # s-slots per PSUM bank
                sx1b = sx1[:].rearrange("p (o g) -> p o g", o=1).broadcast_to(
                    [128, spb, gpm])
                for o in range(uo, uo + uw, 512):
                    nc.tensor.matmul(u2[:, o:o + 512], ident, x1[:, o:o + 512],
                                     start=True, stop=False)
                    nc.tensor.matmul(
                        u2[:, o:o + 512].rearrange("p (s g) -> p s g", s=spb),
                        w2s, sx1b, start=False, stop=True)
                if uo + uw == MC:
                    x1_of.pop(m)

            def stage_f(m, uo, uw):
                u2 = u2_of[m]
                if uo == 0:
                    x2_of[m] = x2p.tile([128, MC], BF16, tag="x2", name="x2")
                x2 = x2_of[m]
                nc.scalar.activation(x2[:, uo:uo + uw], u2[:, uo:uo + uw],
                                     AF.Tanh, bias=b2)
                if uo + uw == MC:
                    u2_of.pop(m)

            def stage_g(m, uo, uw, gpm):
                x2 = x2_of[m]
                so = _SXOFF[m] + uo // S
                tree16(x2[:, uo:uo + uw], sx2all[:, so:so + gpm], gpm, "s2")
                if uo + uw == MC:
                    x2_of.pop(m)

            # software pipeline at layout-unit granularity; tree2 of the
            # previous unit is issued AFTER tree1/u2 of the current one so the
            # in-order DVE queue never blocks the forward chain.
            UNITS = [(m, uo, uw, gpm) for m in range(NMC - 1)
                     for uo, uw, gpm in _units(m)]
            stage_a(1)
            stage_a(2)
            stage_bc(0)
            prev = None
            for k, (m, uo, uw, gpm) in enumerate(UNITS):
                if uo == 0:
                    if m + 3 < NMC:
                        stage_a(m + 3)
                    if m + 1 < NMC:
                        stage_bc(m + 1)
                stage_de(m, uo, uw, gpm)
                if prev is not None:
                    stage_g(*prev)
                    pm = prev[0]
                    if prev[1] + prev[2] == MC:
                        if pm == 3:
                            nc.sync.dma_start(out_d[:, 0:512], sx2all[:, 0:512])
                stage_f(m, uo, uw)
                prev = (m, uo, uw, gpm)
            # last macro: both units' tree1/u2 first (they only need tanh1 and
            # free banks), then tree2(prev) — which waits on tanh2(prev) — so
            # the in-order DVE can't stall the forward chain; both tanh2 units
            # then run back-to-back on ACT and only the tiny trees + out DMA
            # remain after the ACT stream ends.
            lm = NMC - 1
            lus = _units(lm)
            for uo, uw, gpm in lus:
                stage_de(lm, uo, uw, gpm)
            stage_g(*prev)
            nc.sync.dma_start(out_d[:, 512:896], sx2all[:, 512:896])
            for uo, uw, gpm in lus:
                stage_f(lm, uo, uw)
            stage_g(lm, lus[0][0], lus[0][1], lus[0][2])
            nc.sync.dma_start(out_d[:, 896:960], sx2all[:, 896:960])
            stage_g(lm, lus[1][0], lus[1][1], lus[1][2])
            nc.sync.dma_start(out_d[:, 960:OUTPC], sx2all[:, 960:OUTPC])

    nc.compile()
    return nc


def _get_nc():
    if "nc" not in _CACHE:
        _CACHE["nc"] = _build_nc()
    return _CACHE["nc"]


def _pack_block(o4):
    """[gpm, 16, 64] float32 node block -> [128, 16*gpm] bf16 device block."""
    gpm = o4.shape[0]
    top = o4.transpose(2, 1, 0)                        # [OBS, S, gpm]
    sob = o4.sum(axis=1).transpose(1, 0)               # [OBS, gpm]
    bot = np.broadcast_to(sob[:, None, :], top.shape)
    blk = np.concatenate([top, bot], axis=0)           # [128, S, gpm]
    return blk.reshape(128, S * gpm)


def _make_in_maps(cent_obs, w_emb, b_emb, w_gcn, b_gcn):
    w_emb = np.ascontiguousarray(w_emb, np.float32)
    wcomb = np.concatenate(
        [w_emb, (w_emb @ w_gcn[0]) / np.float32(16.0)], axis=0
    )                                                      # [128, HID]
    w2s = w_gcn[1] / np.float32(16.0)                      # [HID, HID]
    ident = np.eye(HID, dtype=np.float32)
    wca = np.ascontiguousarray(wcomb).astype(ml_dtypes.bfloat16)
    wpk = np.concatenate([w2s, ident],
                         axis=1).astype(ml_dtypes.bfloat16)  # [128, 256]
    b1f = (b_gcn[0] + b_emb + b_emb @ w_gcn[0]).astype(np.float32).reshape(HID, 1)
    b2 = b_gcn[1].astype(np.float32).reshape(HID, 1)
    bpk = np.concatenate([b1f, b2], axis=1).astype(np.float32)  # [128, 2]
    shared = {"wca": wca, "wpk": wpk, "bpk": bpk}
    o5 = np.ascontiguousarray(cent_obs, np.float32).reshape(
        N_CORES, NMC, MC // S, S, OBS
    )
    obs_all = np.zeros((N_CORES, NMC, 128, MC), np.float32)
    for m in range(NMC):
        for uo, uw, gpm in _units(m):
            g0 = uo // S
            obs_all[:, m, :, uo:uo + uw] = np.stack(
                [_pack_block(o5[ci, m, g0:g0 + gpm]) for ci in range(N_CORES)]
            )
    obs_all = obs_all.astype(ml_dtypes.bfloat16)
    in_maps = []
    for ci in range(N_CORES):
        m = dict(shared)
        m["obs"] = np.ascontiguousarray(obs_all[ci])
        in_maps.append(m)
    return in_maps


def kernel(cent_obs, w_emb, b_emb, w_gcn, b_gcn, w_fc1, b_fc1,
           edge_src, edge_dst, _trace=False):
    cent_obs = np.asarray(cent_obs, np.float32)
    nc = _get_nc()
    in_maps = _make_in_maps(
        cent_obs, np.asarray(w_emb, np.float32), np.asarray(b_emb, np.float32),
        np.asarray(w_gcn, np.float32), np.asarray(b_gcn, np.float32),
    )
    kw = dict(trace=True) if _trace else {}
    res = run_bass_kernel_spmd(nc, in_maps, list(range(N_CORES)), **kw)
    wfc = (np.asarray(w_fc1, np.float32).reshape(HID) / np.float32(16.0))
    y = np.concatenate(
        [wfc @ np.asarray(res.results[i]["out"]).astype(np.float32)
         for i in range(N_CORES)]
    )
    out = (y + np.float32(np.asarray(b_fc1).reshape(()))).astype(np.float32)
    if _trace:
        _CACHE["last_result"] = res
    return out.reshape(BATCH, 1)
